# revision 14
# baseline (speedup 1.0000x reference)
"""AlphaGCN (3-layer GCN + BN + global mean pool + MLP head) on 8 TRN2 NeuronCores.

Sharding: nodes (and their incident in-edges) are partitioned into 8 contiguous
ranges of 6250. Per layer: each core computes its node-major t = h @ W shard,
scales by dis and writes a bf16 table shard; AllGather builds the full
[50000,128] bf16 table in DRAM; dma_gather fetches each local edge's source row
(edge-major chunks of 128); a 0/1 one-hot matmul scatter-adds messages into
feature-major PSUM groups; BatchNorm stats go through a tiny AllReduce; the
fused scale/shift + ReLU runs on the scalar engine. dis^2*t self term is
implemented as explicit self-loop edges. Pooling = one-hot matmul + AllReduce;
the final MLP is replicated on every core.
"""
import numpy as np
import ml_dtypes

import concourse.bass as bass
import concourse.bacc as bacc
import concourse.tile as tile
import concourse.mybir as mybir
from concourse.bass_utils import run_bass_kernel_spmd

F32 = mybir.dt.float32
BF16 = mybir.dt.bfloat16
I16 = mybir.dt.int16
I32 = mybir.dt.int32
AF = mybir.ActivationFunctionType
ALU = mybir.AluOpType

NCORES = 8
N, E, B = 50000, 600000, 64
F_NODE, H, GF = 64, 128, 7
L = 3
EPS = 1e-5
NLOC = N // NCORES           # 6250
NBLK = (NLOC + 127) // 128   # 49 node blocks; last has 106 rows
LASTP = NLOC - 128 * (NBLK - 1)  # 106
NGRP = (NLOC + 511) // 512   # 13 psum groups of 512 dst cols (last 106)
SPLIT = 32768                # src < SPLIT -> table window A, else window B
BOFF = N - SPLIT             # 17232: window B starts at this row


def _ceil_div(a, b):
    return -(-a // b)


def build_plan(x, edge_index, batch_idx, graph_features, W_emb, b_emb, conv_W,
               conv_b, bn_gamma, bn_beta, gf_W1, gf_b1, gf_W2, gf_b2,
               p_W1, p_b1, p_W2, p_b2, p_W3, p_b3):
    """Host-side numpy prep: edge partition/sort/pad + per-core input tensors."""
    x = np.asarray(x, np.float32)
    src = np.asarray(edge_index[0], np.int64)
    dst = np.asarray(edge_index[1], np.int64)
    batch = np.asarray(batch_idx, np.int64)

    deg = np.bincount(dst, minlength=N).astype(np.float64) + 1.0
    dis = (1.0 / np.sqrt(deg)).astype(np.float32)

    # append self loops (implements dis^2 * t)
    ar = np.arange(N, dtype=np.int64)
    asrc = np.concatenate([src, ar])
    adst = np.concatenate([dst, ar])
    acore = adst // NLOC

    # per-(core, tile, grp) counts
    cnt = np.zeros((NCORES, NBLK, 2), np.int64)
    per_core = []
    for c in range(NCORES):
        m = acore == c
        es = asrc[m]
        edl = adst[m] - c * NLOC
        t = edl >> 7
        grp = (es >= SPLIT).astype(np.int64)
        key = t * 2 + grp
        cnt[c] = np.bincount(key, minlength=2 * NBLK).reshape(NBLK, 2)
        per_core.append((es, edl, t, grp, key))

    # uniform chunk counts across cores
    K = np.zeros((NBLK, 2), np.int64)
    for t in range(NBLK):
        for g in range(2):
            K[t, g] = _ceil_div(int(cnt[:, t, g].max()), 128)

    # call schedule (static, shared): per psum group g, A calls then B calls.
    # dma_gather fails above ~1024 indices/call -> cap calls at 8 chunks.
    MAXCH_CALL = 8
    calls = []
    off_tg = np.zeros((NBLK, 2), np.int64)
    off = 0
    for g in range(NGRP):
        tiles = range(4 * g, min(4 * g + 4, NBLK))
        for grp in (0, 1):
            tl = []  # t_rel per chunk, in stream order
            for t in tiles:
                if K[t, grp] > 0:
                    off_tg[t, grp] = off
                    tl.extend([t - 4 * g] * int(K[t, grp]))
                    off += K[t, grp]
            for s in range(0, len(tl), MAXCH_CALL):
                sub = tl[s: s + MAXCH_CALL]
                calls.append(dict(grp=grp, g=g, tl=sub, nch=len(sub),
                                  off=int(off - len(tl) + s)))
    totc = int(off)

    # per-core streams
    gidx_l, dloc_l = [], []
    for c in range(NCORES):
        es, edl, t, grp, key = per_core[c]
        order = np.argsort(key, kind="stable")
        es_s, edl_s, t_s, grp_s = es[order], edl[order], t[order], grp[order]
        idxv = np.where(grp_s == 0, es_s, es_s - BOFF).astype(np.int16)
        dlv = (edl_s - t_s * 128).astype(np.float32)
        bounds = np.zeros(2 * NBLK + 1, np.int64)
        bounds[1:] = np.cumsum(cnt[c].reshape(-1))
        sidx = np.zeros(totc * 128, np.int16)
        sdl = np.full(totc * 128, -1.0, np.float32)
        for t2 in range(NBLK):
            for g2 in range(2):
                k2 = t2 * 2 + g2
                lo, hi = bounds[k2], bounds[k2 + 1]
                n = hi - lo
                if n == 0:
                    continue
                p0 = off_tg[t2, g2] * 128
                sidx[p0:p0 + n] = idxv[lo:hi]
                sdl[p0:p0 + n] = dlv[lo:hi]
        gidx_l.append(np.tile(sidx.reshape(-1, 16).T, (8, 1)).copy())
        dloc_l.append(np.ascontiguousarray(sdl.reshape(totc, 128).T))

    # per-core node-major helpers
    xT_l, disnm_l, disrow_l, gblk_l = [], [], [], []
    for c in range(NCORES):
        lo = c * NLOC
        xT_l.append(np.ascontiguousarray(x[lo:lo + NLOC].T))
        d = np.zeros((128, NBLK), np.float32)
        gb = np.full((128, NBLK), -1.0, np.float32)
        dseg = dis[lo:lo + NLOC]
        bseg = batch[lo:lo + NLOC].astype(np.float32)
        for nb in range(NBLK):
            pn = 128 if nb < NBLK - 1 else LASTP
            d[:pn, nb] = dseg[nb * 128: nb * 128 + pn]
            gb[:pn, nb] = bseg[nb * 128: nb * 128 + pn]
        disnm_l.append(d)
        disrow_l.append(dseg.reshape(1, NLOC).copy())
        gblk_l.append(gb)

    counts = np.bincount(batch, minlength=B).astype(np.float32)
    invcnt = (1.0 / np.maximum(counts, 1.0)).reshape(B, 1).astype(np.float32)

    conv_W = np.asarray(conv_W, np.float32)
    weights = dict(
        wemb=np.asarray(W_emb, np.float32),
        bemb=np.asarray(b_emb, np.float32).reshape(H, 1),
        convw=np.ascontiguousarray(conv_W.transpose(1, 0, 2).reshape(H, L * H)),
        bng=np.ascontiguousarray(np.asarray(bn_gamma, np.float32).T),
        bnb=np.ascontiguousarray(np.asarray(bn_beta, np.float32).T),
        gft=np.ascontiguousarray(np.asarray(graph_features, np.float32).T),
        gfw1=np.asarray(gf_W1, np.float32),
        gfb1=np.asarray(gf_b1, np.float32).reshape(-1, 1),
        gfw2=np.asarray(gf_W2, np.float32),
        gfb2=np.asarray(gf_b2, np.float32).reshape(-1, 1),
        pw1a=np.ascontiguousarray(np.asarray(p_W1, np.float32)[:H]),
        pw1b=np.ascontiguousarray(np.asarray(p_W1, np.float32)[H:]),
        pb1=np.asarray(p_b1, np.float32).reshape(-1, 1),
        pw2=np.asarray(p_W2, np.float32),
        pb2=np.asarray(p_b2, np.float32).reshape(-1, 1),
        pw3=np.asarray(p_W3, np.float32),
        pb3=np.asarray(p_b3, np.float32).reshape(1, 1),
        invcnt=invcnt,
    )

    in_maps = []
    for c in range(NCORES):
        m = dict(weights)
        m["xt"] = xT_l[c]
        m["gidx"] = gidx_l[c]
        m["dloc"] = dloc_l[c]
        m["disnm"] = disnm_l[c]
        m["disrow"] = disrow_l[c]
        m["gblk"] = gblk_l[c]
        in_maps.append(m)

    plan = dict(calls=calls, totc=totc,
                maxch=max(cl["nch"] for cl in calls))
    return plan, in_maps


def build_nc(plan, reps=1, debug_taps=False):
    calls = plan["calls"]
    totc = plan["totc"]
    maxch = plan["maxch"]

    nc = bacc.Bacc(num_devices=NCORES, target_bir_lowering=False, debug=False)
    P = {}
    for name, shape, dt in [
        ("xt", [F_NODE, NLOC], F32), ("gidx", [128, totc * 8], I16),
        ("dloc", [128, totc], F32), ("disnm", [128, NBLK], F32),
        ("disrow", [1, NLOC], F32), ("gblk", [128, NBLK], F32),
        ("wemb", [F_NODE, H], F32), ("bemb", [H, 1], F32),
        ("convw", [H, L * H], F32), ("bng", [H, L], F32), ("bnb", [H, L], F32),
        ("gft", [GF, B], F32), ("gfw1", [GF, H // 2], F32),
        ("gfb1", [H // 2, 1], F32), ("gfw2", [H // 2, H // 4], F32),
        ("gfb2", [H // 4, 1], F32), ("pw1a", [H, H // 2], F32),
        ("pw1b", [H // 4, H // 2], F32), ("pb1", [H // 2, 1], F32),
        ("pw2", [H // 2, H // 4], F32), ("pb2", [H // 4, 1], F32),
        ("pw3", [H // 4, 1], F32), ("pb3", [1, 1], F32),
        ("invcnt", [B, 1], F32),
    ]:
        P[name] = nc.declare_dram_parameter(name, shape, dt, isOutput=False)
    out_p = nc.declare_dram_parameter("out", [B, 1], F32, isOutput=True)
    taps = {}
    if debug_taps:
        for i in range(L):
            taps[i] = nc.declare_dram_parameter(f"h_tap{i}", [H, NLOC], F32,
                                                isOutput=True)

    with tile.TileContext(nc) as tc:
        with (
            tc.tile_pool(name="perm", bufs=1) as perm,
            tc.tile_pool(name="dram", bufs=1, space="DRAM") as dram,
        ):
            # ---- persistent SBUF tiles
            sb = {}
            for name, shape, dt in [
                ("gidx", [128, totc * 8], I16), ("dloc", [128, totc], F32),
                ("disnm", [128, NBLK], F32), ("gblk", [128, NBLK], F32),
                ("wemb", [F_NODE, H], F32), ("bemb", [H, 1], F32),
                ("convw", [H, L * H], F32), ("bng", [H, L], F32),
                ("bnb", [H, L], F32), ("gft", [GF, B], F32),
                ("gfw1", [GF, H // 2], F32), ("gfb1", [H // 2, 1], F32),
                ("gfw2", [H // 2, H // 4], F32), ("gfb2", [H // 4, 1], F32),
                ("pw1a", [H, H // 2], F32), ("pw1b", [H // 4, H // 2], F32),
                ("pb1", [H // 2, 1], F32), ("pw2", [H // 2, H // 4], F32),
                ("pb2", [H // 4, 1], F32), ("pw3", [H // 4, 1], F32),
                ("pb3", [1, 1], F32), ("invcnt", [B, 1], F32),
            ]:
                sb[name] = perm.tile(shape, dt, name=f"sb_{name}")
                nc.sync.dma_start(sb[name][:], P[name][:])

            h = perm.tile([H, NLOC], F32, name="h")
            xacc = perm.tile([H, NLOC], F32, name="xacc")
            disb = perm.tile([H, NLOC], F32, name="disb")
            stage = perm.tile([128, NBLK, H], BF16, name="stage")
            iota_i = perm.tile([128, 128], I32, name="iota_i")
            iotaf = perm.tile([128, 128], F32, name="iotaf")
            iotap_i = perm.tile([128, 128], I32, name="iotap_i")
            ident = perm.tile([128, 128], F32, name="ident")
            ones1 = perm.tile([1, 128], F32, name="ones1")
            s1p = perm.tile([128, NGRP], F32, name="s1p")
            s2p = perm.tile([128, NGRP], F32, name="s2p")
            sred = perm.tile([128, 2], F32, name="sred")
            statr = perm.tile([128, 2], F32, name="statr")
            scr = perm.tile([128, 512], F32, name="scr")
            scr2 = perm.tile([128, 512], F32, name="scr2")
            bnc = perm.tile([128, 8], F32, name="bnc")  # mu,m2,musq,var,std,inv,scale,shift
            h3nm = perm.tile([128, 128], F32, name="h3nm", bufs=4)
            gsum_sb = perm.tile([B, H], F32, name="gsum_sb")
            gembT = perm.tile([H, B], F32, name="gembT")
            outsb = perm.tile([1, B], F32, name="outsb")

            # DRAM bounce tiles (Shared collective outputs need unique writers)
            ag_in = dram.tile([NLOC, H], BF16, name="ag_in")
            tables = [[dram.tile([N, H], BF16, addr_space="Shared",
                                 name=f"table_r{r}l{i}") for i in range(L)]
                      for r in range(reps)]
            stats_in = dram.tile([128, 2], F32, name="stats_in")
            stats_outs = [[dram.tile([128, 2], F32, addr_space="Shared",
                                     name=f"stats_out_r{r}l{i}")
                           for i in range(L)] for r in range(reps)]
            gsum_in = dram.tile([B, H], F32, name="gsum_in")
            gsum_outs = [dram.tile([B, H], F32, addr_space="Shared",
                                   name=f"gsum_out_r{r}") for r in range(reps)]

            # ---- constants: iota row, identity, dis broadcast
            nc.gpsimd.iota(iota_i[:], pattern=[[1, 128]], base=0,
                           channel_multiplier=0)
            nc.vector.tensor_copy(iotaf[:], iota_i[:])
            nc.gpsimd.iota(iotap_i[:], pattern=[[0, 128]], base=0,
                           channel_multiplier=1)
            itp_f = perm.tile([128, 128], F32, name="itp_f")
            nc.vector.tensor_copy(itp_f[:], iotap_i[:])
            nc.vector.tensor_tensor(out=ident[:], in0=itp_f[:], in1=iotaf[:],
                                    op=ALU.is_equal)
            nc.vector.memset(ones1[:], 1.0)
            disrow_sb = perm.tile([1, NLOC], F32, name="disrow_sb")
            nc.sync.dma_start(disrow_sb[:], P["disrow"][:])

            with tc.tile_pool(name="psB", bufs=2, space="PSUM") as psB:
                for g in range(NGRP):
                    w = 512 if g < NGRP - 1 else LASTP
                    pt = psB.tile([128, 512], F32, tag="pbc")
                    nc.tensor.matmul(pt[:, :w], lhsT=ones1[:],
                                     rhs=disrow_sb[:, g * 512: g * 512 + w],
                                     start=True, stop=True)
                    nc.vector.tensor_copy(disb[:, g * 512: g * 512 + w],
                                          pt[:, :w])

            for rep in range(reps):
                # ---- embed: h = relu(W_emb^T @ xT + b_emb)
                with (
                    tc.tile_pool(name="psE", bufs=2, space="PSUM") as psE,
                    tc.tile_pool(name="sbE", bufs=3) as sbE,
                ):
                    for g in range(NGRP):
                        w = 512 if g < NGRP - 1 else LASTP
                        xch = sbE.tile([F_NODE, 512], F32, tag="xch")
                        nc.sync.dma_start(xch[:, :w],
                                          P["xt"][:, g * 512: g * 512 + w])
                        pt = psE.tile([128, 512], F32, tag="pse")
                        nc.tensor.matmul(pt[:, :w], lhsT=sb["wemb"][:],
                                         rhs=xch[:, :w],
                                         start=True, stop=True)
                        nc.scalar.activation(h[:, g * 512: g * 512 + w],
                                             pt[:, :w], AF.Relu,
                                             bias=sb["bemb"][:], scale=1.0)

                # ---- GCN layers
                for li in range(L):
                    # t = h @ W (node-major), scale by dis, cast bf16 -> stage
                    with tc.tile_pool(name=f"psT{li}", bufs=4, space="PSUM") as psT:
                        for nb in range(NBLK):
                            pn = 128 if nb < NBLK - 1 else LASTP
                            pt = psT.tile([128, H], F32, tag="pst")
                            nc.tensor.matmul(
                                pt[:pn, :],
                                lhsT=h[:, nb * 128: nb * 128 + pn],
                                rhs=sb["convw"][:, li * H: (li + 1) * H],
                                start=True, stop=True)
                            nc.scalar.activation(
                                stage[:pn, nb, :], pt[:pn, :], AF.Copy,
                                bias=0.0, scale=sb["disnm"][:pn, nb: nb + 1])
                    nc.sync.dma_start(
                        ag_in[0: 128 * (NBLK - 1), :].rearrange(
                            "(nb p) f -> p nb f", p=128),
                        stage[:, 0: NBLK - 1, :])
                    nc.sync.dma_start(ag_in[128 * (NBLK - 1): NLOC, :],
                                      stage[:LASTP, NBLK - 1, :])
                    table = tables[rep][li]
                    nc.gpsimd.collective_compute(
                        "AllGather", ALU.bypass,
                        replica_groups=[list(range(NCORES))],
                        ins=[ag_in[:].opt()], outs=[table[:].opt()])

                    # scatter phase
                    with (
                        tc.tile_pool(name=f"msgp{li}", bufs=4) as msgp,
                        tc.tile_pool(name=f"psS{li}", bufs=2, space="PSUM") as psS,
                    ):
                        ci = 0
                        for g in range(NGRP):
                            w = 512 if g < NGRP - 1 else LASTP
                            psg = psS.tile([128, 512], F32, tag="psg")
                            nc.vector.memset(psg[:], 0.0)
                            while ci < len(calls) and calls[ci]["g"] == g:
                                cl = calls[ci]
                                nch = cl["nch"]
                                off = cl["off"]
                                last_call = (ci + 1 == len(calls)
                                             or calls[ci + 1]["g"] != g)
                                msg = msgp.tile([128, maxch, H], BF16, tag="msg")
                                src_ap = (table[0:SPLIT, :] if cl["grp"] == 0
                                          else table[BOFF:N, :])
                                nc.gpsimd.dma_gather(
                                    out_ap=msg[:, :nch, :], in_ap=src_ap,
                                    idxs_ap=sb["gidx"][:, off * 8: (off + nch) * 8],
                                    num_idxs=nch * 128, num_idxs_reg=nch * 128,
                                    elem_size=H)
                                oh = msgp.tile([128, maxch, 128], BF16, tag="oh")
                                d_b = sb["dloc"][:, off: off + nch].unsqueeze(
                                    2).broadcast_to([128, nch, 128])
                                i_b = iotaf[:].unsqueeze(1).broadcast_to(
                                    [128, nch, 128])
                                nc.vector.tensor_tensor(
                                    out=oh[:, :nch, :], in0=d_b, in1=i_b,
                                    op=ALU.is_equal)
                                for cc, t_rel in enumerate(cl["tl"]):
                                    nc.tensor.matmul(
                                        psg[:, t_rel * 128: (t_rel + 1) * 128],
                                        lhsT=msg[:, cc, :],
                                        rhs=oh[:, cc, :],
                                        start=False,
                                        stop=(last_call and cc == nch - 1),
                                        skip_group_check=True)
                                ci += 1
                            # epilogue: x = psum * dis ; stats
                            cols = slice(g * 512, g * 512 + w)
                            nc.vector.tensor_tensor(out=xacc[:, cols],
                                                    in0=psg[:, :w],
                                                    in1=disb[:, cols],
                                                    op=ALU.mult)
                            nc.vector.tensor_reduce(
                                s1p[:, g: g + 1], xacc[:, cols],
                                axis=mybir.AxisListType.X, op=ALU.add)
                            nc.scalar.activation(scr[:, :w], xacc[:, cols],
                                                 AF.Square,
                                                 accum_out=s2p[:, g: g + 1])
                    # BN stats allreduce
                    nc.vector.tensor_reduce(sred[:, 0:1], s1p[:],
                                            axis=mybir.AxisListType.X, op=ALU.add)
                    nc.vector.tensor_reduce(sred[:, 1:2], s2p[:],
                                            axis=mybir.AxisListType.X, op=ALU.add)
                    nc.sync.dma_start(stats_in[:], sred[:])
                    stats_out = stats_outs[rep][li]
                    nc.gpsimd.collective_compute(
                        "AllReduce", ALU.add,
                        replica_groups=[list(range(NCORES))],
                        ins=[stats_in[:].opt()], outs=[stats_out[:].opt()])
                    nc.sync.dma_start(statr[:], stats_out[:])
                    # scale/shift
                    nc.vector.tensor_scalar_mul(bnc[:, 0:1], statr[:, 0:1], 1.0 / N)
                    nc.vector.tensor_scalar_mul(bnc[:, 1:2], statr[:, 1:2], 1.0 / N)
                    nc.vector.tensor_tensor(out=bnc[:, 2:3], in0=bnc[:, 0:1],
                                            in1=bnc[:, 0:1], op=ALU.mult)
                    nc.vector.tensor_tensor(out=bnc[:, 3:4], in0=bnc[:, 1:2],
                                            in1=bnc[:, 2:3], op=ALU.subtract)
                    nc.vector.tensor_scalar_add(bnc[:, 3:4], bnc[:, 3:4], EPS)
                    nc.scalar.sqrt(bnc[:, 4:5], bnc[:, 3:4])
                    nc.vector.reciprocal(bnc[:, 5:6], bnc[:, 4:5])
                    nc.vector.tensor_tensor(out=bnc[:, 6:7],
                                            in0=sb["bng"][:, li: li + 1],
                                            in1=bnc[:, 5:6], op=ALU.mult)
                    nc.vector.tensor_tensor(out=bnc[:, 7:8], in0=bnc[:, 0:1],
                                            in1=bnc[:, 6:7], op=ALU.mult)
                    nc.vector.tensor_tensor(out=bnc[:, 7:8],
                                            in0=sb["bnb"][:, li: li + 1],
                                            in1=bnc[:, 7:8], op=ALU.subtract)
                    # apply BN + relu (+ residual)
                    for g in range(NGRP):
                        w = 512 if g < NGRP - 1 else LASTP
                        cols = slice(g * 512, g * 512 + w)
                        if li == 0:
                            nc.scalar.activation(h[:, cols], xacc[:, cols],
                                                 AF.Relu, bias=bnc[:, 7:8],
                                                 scale=bnc[:, 6:7])
                        else:
                            nc.scalar.activation(scr2[:, :w], xacc[:, cols],
                                                 AF.Relu, bias=bnc[:, 7:8],
                                                 scale=bnc[:, 6:7])
                            nc.vector.tensor_tensor(out=h[:, cols],
                                                    in0=scr2[:, :w],
                                                    in1=h[:, cols], op=ALU.add)
                    if debug_taps:
                        nc.sync.dma_start(taps[li][:], h[:])

                # ---- global mean pool
                with (
                    tc.tile_pool(name="psP", bufs=4, space="PSUM") as psP,
                    tc.tile_pool(name="sbP", bufs=1) as sbP,
                ):
                    ohb = sbP.tile([128, NBLK, B], F32, tag="ohb")
                    gb_b = sb["gblk"][:].unsqueeze(2).broadcast_to(
                        [128, NBLK, B])
                    io_b = iotaf[:, :B].unsqueeze(1).broadcast_to(
                        [128, NBLK, B])
                    nc.vector.tensor_tensor(out=ohb[:], in0=gb_b, in1=io_b,
                                            op=ALU.is_equal)
                    gps = psP.tile([B, H], F32, tag="gps", bufs=1)
                    nc.vector.memset(gps[:], 0.0)
                    for nb in range(NBLK):
                        pn = 128 if nb < NBLK - 1 else LASTP
                        tr = psP.tile([128, 128], F32, tag="ptr")
                        nc.tensor.transpose(tr[:pn, :],
                                            h[:, nb * 128: nb * 128 + pn],
                                            ident[:])
                        nc.scalar.activation(h3nm[:pn, :], tr[:pn, :], AF.Copy,
                                             bias=0.0, scale=1.0)
                        nc.tensor.matmul(gps[:], lhsT=ohb[:pn, nb, :],
                                         rhs=h3nm[:pn, :], start=False,
                                         stop=(nb == NBLK - 1),
                                         skip_group_check=True)
                    nc.scalar.activation(gsum_sb[:], gps[:], AF.Copy,
                                         bias=0.0, scale=1.0)
                nc.sync.dma_start(gsum_in[:], gsum_sb[:])
                gsum_out = gsum_outs[rep]
                nc.gpsimd.collective_compute(
                    "AllReduce", ALU.add, replica_groups=[list(range(NCORES))],
                    ins=[gsum_in[:].opt()], outs=[gsum_out[:].opt()])
                gsumr = perm.tile([B, H], F32, name=f"gsumr{rep}", tag="gsumr")
                nc.sync.dma_start(gsumr[:], gsum_out[:])

                # ---- final MLP (replicated on every core)
                with tc.tile_pool(name="psM", bufs=1, space="PSUM") as psM:
                    gemb = perm.tile([B, H], F32, name=f"gemb{rep}", tag="gemb")
                    nc.vector.tensor_scalar_mul(gemb[:], gsumr[:],
                                                sb["invcnt"][:])
                    ptr = psM.tile([H, B], F32, tag="mtr")
                    nc.tensor.transpose(ptr[:], gemb[:], ident[:B, :B])
                    nc.scalar.activation(gembT[:], ptr[:], AF.Copy, bias=0.0,
                                         scale=1.0)
                    # gf path
                    p1 = psM.tile([H // 2, B], F32, tag="m1")
                    nc.tensor.matmul(p1[:], lhsT=sb["gfw1"][:], rhs=sb["gft"][:],
                                     start=True, stop=True)
                    g1s = perm.tile([H // 2, B], F32, name=f"g1s{rep}", tag="g1s")
                    nc.scalar.activation(g1s[:], p1[:], AF.Relu,
                                         bias=sb["gfb1"][:], scale=1.0)
                    p2 = psM.tile([H // 4, B], F32, tag="m2")
                    nc.tensor.matmul(p2[:], lhsT=sb["gfw2"][:], rhs=g1s[:],
                                     start=True, stop=True)
                    g2s = perm.tile([H // 4, B], F32, name=f"g2s{rep}", tag="g2s")
                    nc.scalar.activation(g2s[:], p2[:], AF.Identity,
                                         bias=sb["gfb2"][:], scale=1.0)
                    # head
                    q1 = psM.tile([H // 2, B], F32, tag="m3")
                    nc.tensor.matmul(q1[:], lhsT=sb["pw1a"][:], rhs=gembT[:],
                                     start=True, stop=False,
                                     skip_group_check=True)
                    nc.tensor.matmul(q1[:], lhsT=sb["pw1b"][:], rhs=g2s[:],
                                     start=False, stop=True,
                                     skip_group_check=True)
                    q1s = perm.tile([H // 2, B], F32, name=f"q1s{rep}", tag="q1s")
                    nc.scalar.activation(q1s[:], q1[:], AF.Relu,
                                         bias=sb["pb1"][:], scale=1.0)
                    q2 = psM.tile([H // 4, B], F32, tag="m4")
                    nc.tensor.matmul(q2[:], lhsT=sb["pw2"][:], rhs=q1s[:],
                                     start=True, stop=True)
                    q2s = perm.tile([H // 4, B], F32, name=f"q2s{rep}", tag="q2s")
                    nc.scalar.activation(q2s[:], q2[:], AF.Relu,
                                         bias=sb["pb2"][:], scale=1.0)
                    q3 = psM.tile([1, B], F32, tag="m5")
                    nc.tensor.matmul(q3[:], lhsT=sb["pw3"][:], rhs=q2s[:],
                                     start=True, stop=True)
                    nc.scalar.activation(outsb[:], q3[:], AF.Identity,
                                         bias=sb["pb3"][:], scale=1.0)
                nc.sync.dma_start(out_p[:].rearrange("b o -> o b"), outsb[:])

    nc.compile()
    return nc


def kernel(**inputs):
    plan, in_maps = build_plan(**inputs)
    nc = build_nc(plan)
    res = run_bass_kernel_spmd(nc, in_maps, core_ids=list(range(NCORES)),
                               trace=False)
    return np.asarray(res.results[0]["out"], np.float32)


# revision 23
# speedup vs baseline: 27.0406x; 27.0406x over previous
"""AlphaGCN (3-layer GCN + BN + global mean pool + MLP head) on 8 TRN2 NeuronCores.

Sharding: nodes (and their incident in-edges) are partitioned into 8 contiguous
ranges of 6250. Per layer: each core computes its node-major t = h @ W shard,
scales by dis and writes a bf16 table shard; AllGather builds the full
[50000,128] bf16 table in DRAM; dma_gather fetches each local edge's source row
(edge-major chunks of 128); a 0/1 one-hot matmul scatter-adds messages into
feature-major PSUM groups; BatchNorm stats go through a tiny AllReduce; the
fused scale/shift + ReLU runs on the scalar engine. dis^2*t self term is
implemented as explicit self-loop edges. Pooling = one-hot matmul + AllReduce;
the final MLP is replicated on every core.
"""
import numpy as np
import ml_dtypes

import concourse.bass as bass
import concourse.bacc as bacc
import concourse.tile as tile
import concourse.mybir as mybir
from concourse.bass_utils import run_bass_kernel_spmd

F32 = mybir.dt.float32
BF16 = mybir.dt.bfloat16
I16 = mybir.dt.int16
I32 = mybir.dt.int32
AF = mybir.ActivationFunctionType
ALU = mybir.AluOpType

NCORES = 8
N, E, B = 50000, 600000, 64
F_NODE, H, GF = 64, 128, 7
L = 3
EPS = 1e-5
NLOC = N // NCORES           # 6250
NBLK = (NLOC + 127) // 128   # 49 node blocks; last has 106 rows
LASTP = NLOC - 128 * (NBLK - 1)  # 106
NGRP = (NLOC + 511) // 512   # 13 psum groups of 512 dst cols (last 106)
SPLIT = 32768                # src < SPLIT -> table window A, else window B
BOFF = N - SPLIT             # 17232: window B starts at this row


def _ceil_div(a, b):
    return -(-a // b)


def build_plan(x, edge_index, batch_idx, graph_features, W_emb, b_emb, conv_W,
               conv_b, bn_gamma, bn_beta, gf_W1, gf_b1, gf_W2, gf_b2,
               p_W1, p_b1, p_W2, p_b2, p_W3, p_b3):
    """Host-side numpy prep: edge partition/sort/pad + per-core input tensors."""
    x = np.asarray(x, np.float32)
    src = np.asarray(edge_index[0], np.int64)
    dst = np.asarray(edge_index[1], np.int64)
    batch = np.asarray(batch_idx, np.int64)

    deg = np.bincount(dst, minlength=N).astype(np.float64) + 1.0
    dis = (1.0 / np.sqrt(deg)).astype(np.float32)

    # append self loops (implements dis^2 * t)
    ar = np.arange(N, dtype=np.int64)
    asrc = np.concatenate([src, ar])
    adst = np.concatenate([dst, ar])
    acore = adst // NLOC

    # per-(core, tile, grp) counts
    cnt = np.zeros((NCORES, NBLK, 2), np.int64)
    per_core = []
    for c in range(NCORES):
        m = acore == c
        es = asrc[m]
        edl = adst[m] - c * NLOC
        t = edl >> 7
        grp = (es >= SPLIT).astype(np.int64)
        key = t * 2 + grp
        cnt[c] = np.bincount(key, minlength=2 * NBLK).reshape(NBLK, 2)
        per_core.append((es, edl, t, grp, key))

    # uniform chunk counts across cores
    K = np.zeros((NBLK, 2), np.int64)
    for t in range(NBLK):
        for g in range(2):
            K[t, g] = _ceil_div(int(cnt[:, t, g].max()), 128)

    # call schedule (static, shared): per psum group g, A calls then B calls.
    # dma_gather fails above ~1024 indices/call -> cap calls at 8 chunks.
    MAXCH_CALL = 8
    calls = []
    off_tg = np.zeros((NBLK, 2), np.int64)
    off = 0
    for g in range(NGRP):
        tiles = range(4 * g, min(4 * g + 4, NBLK))
        for grp in (0, 1):
            tl = []  # t_rel per chunk, in stream order
            for t in tiles:
                if K[t, grp] > 0:
                    off_tg[t, grp] = off
                    tl.extend([t - 4 * g] * int(K[t, grp]))
                    off += K[t, grp]
            for s in range(0, len(tl), MAXCH_CALL):
                sub = tl[s: s + MAXCH_CALL]
                calls.append(dict(grp=grp, g=g, tl=sub, nch=len(sub),
                                  off=int(off - len(tl) + s)))
    totc = int(off)

    # per-core streams
    gidx_l, dloc_l = [], []
    for c in range(NCORES):
        es, edl, t, grp, key = per_core[c]
        order = np.argsort(key, kind="stable")
        es_s, edl_s, t_s, grp_s = es[order], edl[order], t[order], grp[order]
        idxv = np.where(grp_s == 0, es_s, es_s - BOFF).astype(np.int16)
        dlv = (edl_s - t_s * 128).astype(np.float32)
        bounds = np.zeros(2 * NBLK + 1, np.int64)
        bounds[1:] = np.cumsum(cnt[c].reshape(-1))
        sidx = np.zeros(totc * 128, np.int16)
        sdl = np.full(totc * 128, -1.0, np.float32)
        for t2 in range(NBLK):
            for g2 in range(2):
                k2 = t2 * 2 + g2
                lo, hi = bounds[k2], bounds[k2 + 1]
                n = hi - lo
                if n == 0:
                    continue
                p0 = off_tg[t2, g2] * 128
                sidx[p0:p0 + n] = idxv[lo:hi]
                sdl[p0:p0 + n] = dlv[lo:hi]
        gidx_l.append(np.tile(sidx.reshape(-1, 16).T, (8, 1)).copy())
        dloc_l.append(np.ascontiguousarray(sdl.reshape(totc, 128).T))

    # per-core node-major helpers
    xT_l, disnm_l, disrow_l, gblk_l = [], [], [], []
    for c in range(NCORES):
        lo = c * NLOC
        xT_l.append(np.ascontiguousarray(x[lo:lo + NLOC].T))
        d = np.zeros((128, NBLK), np.float32)
        gb = np.full((128, NBLK), -1.0, np.float32)
        dseg = dis[lo:lo + NLOC]
        bseg = batch[lo:lo + NLOC].astype(np.float32)
        for nb in range(NBLK):
            pn = 128 if nb < NBLK - 1 else LASTP
            d[:pn, nb] = dseg[nb * 128: nb * 128 + pn]
            gb[:pn, nb] = bseg[nb * 128: nb * 128 + pn]
        disnm_l.append(d)
        disrow_l.append(dseg.reshape(1, NLOC).copy())
        gblk_l.append(gb)

    counts = np.bincount(batch, minlength=B).astype(np.float32)
    invcnt = (1.0 / np.maximum(counts, 1.0)).reshape(B, 1).astype(np.float32)

    conv_W = np.asarray(conv_W, np.float32)
    weights = dict(
        wemb=np.asarray(W_emb, np.float32),
        bemb=np.asarray(b_emb, np.float32).reshape(H, 1),
        convw=np.ascontiguousarray(conv_W.transpose(1, 0, 2).reshape(H, L * H)),
        bng=np.ascontiguousarray(np.asarray(bn_gamma, np.float32).T),
        bnb=np.ascontiguousarray(np.asarray(bn_beta, np.float32).T),
        gft=np.ascontiguousarray(np.asarray(graph_features, np.float32).T),
        gfw1=np.asarray(gf_W1, np.float32),
        gfb1=np.asarray(gf_b1, np.float32).reshape(-1, 1),
        gfw2=np.asarray(gf_W2, np.float32),
        gfb2=np.asarray(gf_b2, np.float32).reshape(-1, 1),
        pw1a=np.ascontiguousarray(np.asarray(p_W1, np.float32)[:H]),
        pw1b=np.ascontiguousarray(np.asarray(p_W1, np.float32)[H:]),
        pb1=np.asarray(p_b1, np.float32).reshape(-1, 1),
        pw2=np.asarray(p_W2, np.float32),
        pb2=np.asarray(p_b2, np.float32).reshape(-1, 1),
        pw3=np.asarray(p_W3, np.float32),
        pb3=np.asarray(p_b3, np.float32).reshape(1, 1),
        invcnt=invcnt,
    )

    # pack all f32 tensors into one flat param (fewer PJRT args = less
    # dispatch overhead); int16 gidx stays separate
    offs = {}
    in_maps = []
    for c in range(NCORES):
        per = dict(weights)
        per["xt"] = xT_l[c]
        per["dloc"] = dloc_l[c]
        per["disnm"] = disnm_l[c]
        per["disrow"] = disrow_l[c]
        per["gblk"] = gblk_l[c]
        parts = []
        off = 0
        for name in sorted(per):
            a = np.ascontiguousarray(per[name], dtype=np.float32)
            if c == 0:
                offs[name] = (off, tuple(a.shape))
            parts.append(a.reshape(-1))
            off += a.size
        in_maps.append({"pf32": np.concatenate(parts).reshape(1, -1),
                        "gidx": gidx_l[c]})

    plan = dict(calls=calls, totc=totc, offs=offs,
                nf32=int(in_maps[0]["pf32"].size),
                maxch=max(cl["nch"] for cl in calls))
    return plan, in_maps


def build_nc(plan, reps=1, debug_taps=False, sim_mode=False, skip=()):
    calls = plan["calls"]
    totc = plan["totc"]
    maxch = plan["maxch"]

    offs = plan["offs"]

    nc = bacc.Bacc(num_devices=NCORES, target_bir_lowering=False, debug=False,
                   detect_race_conditions=not (sim_mode and skip))
    pf32 = nc.declare_dram_parameter("pf32", [1, plan["nf32"]], F32,
                                     isOutput=False)
    gidx_p = nc.declare_dram_parameter("gidx", [128, totc * 8], I16,
                                       isOutput=False)
    out_p = nc.declare_dram_parameter("out", [B, 1], F32, isOutput=True)

    def pview(name):
        off, shape = offs[name]
        r = 1 if len(shape) == 1 else int(np.prod(shape[:-1]))
        c = int(shape[-1])
        return pf32[0:1, off: off + r * c].rearrange("o (r c) -> (o r) c", c=c)
    taps = {}
    if debug_taps:
        for i in range(L):
            taps[i] = nc.declare_dram_parameter(f"h_tap{i}", [H, NLOC], F32,
                                                isOutput=True)

    with tile.TileContext(nc) as tc:
        if sim_mode and skip:
            tc.race_detector_enabled = False
        with (
            tc.tile_pool(name="perm", bufs=1) as perm,
            tc.tile_pool(name="dram", bufs=1, space="DRAM") as dram,
        ):
            # ---- persistent SBUF tiles
            sb = {}
            sb["gidx"] = perm.tile([128, totc * 8], I16, name="sb_gidx")
            nc.sync.dma_start(sb["gidx"][:], gidx_p[:])
            for name, shape in [
                ("dloc", [128, totc]),
                ("disnm", [128, NBLK]), ("gblk", [128, NBLK]),
                ("wemb", [F_NODE, H]), ("bemb", [H, 1]),
                ("convw", [H, L * H]), ("bng", [H, L]),
                ("bnb", [H, L]), ("gft", [GF, B]),
                ("gfw1", [GF, H // 2]), ("gfb1", [H // 2, 1]),
                ("gfw2", [H // 2, H // 4]), ("gfb2", [H // 4, 1]),
                ("pw1a", [H, H // 2]), ("pw1b", [H // 4, H // 2]),
                ("pb1", [H // 2, 1]), ("pw2", [H // 2, H // 4]),
                ("pb2", [H // 4, 1]), ("pw3", [H // 4, 1]),
                ("pb3", [1, 1]), ("invcnt", [B, 1]),
            ]:
                assert tuple(offs[name][1]) == tuple(shape), (name, offs[name])
                sb[name] = perm.tile(shape, F32, name=f"sb_{name}")
                nc.sync.dma_start(sb[name][:], pview(name))

            h = perm.tile([H, NLOC], F32, name="h")
            xacc = perm.tile([H, NLOC], F32, name="xacc")
            disb = perm.tile([H, NLOC], F32, name="disb")
            stage = perm.tile([128, NBLK, H], BF16, name="stage")
            iota_i = perm.tile([128, 128], I32, name="iota_i")
            iotaf = perm.tile([128, 128], F32, name="iotaf")
            iotap_i = perm.tile([128, 128], I32, name="iotap_i")
            ident = perm.tile([128, 128], F32, name="ident")
            ones1 = perm.tile([1, 128], F32, name="ones1")
            s1p = perm.tile([128, NGRP], F32, name="s1p")
            s2p = perm.tile([128, NGRP], F32, name="s2p")
            sred = perm.tile([128, 2], F32, name="sred")
            statr = perm.tile([128, 2], F32, name="statr")
            scr = perm.tile([128, 512], F32, name="scr")
            scr2 = perm.tile([128, 512], F32, name="scr2")
            bnc = perm.tile([128, 8], F32, name="bnc")  # mu,m2,musq,var,std,inv,scale,shift
            h3nm = perm.tile([128, 128], F32, name="h3nm", bufs=4)
            gsum_sb = perm.tile([B, H], F32, name="gsum_sb")
            gembT = perm.tile([H, B], F32, name="gembT")
            outsb = perm.tile([1, B], F32, name="outsb")

            # DRAM bounce tiles (Shared collective outputs need unique writers)
            ag_in = dram.tile([NLOC, H], BF16, name="ag_in")
            tables = [[dram.tile([N, H], BF16, addr_space="Shared",
                                 name=f"table_r{r}l{i}") for i in range(L)]
                      for r in range(reps)]
            stats_in = dram.tile([128, 2], F32, name="stats_in")
            stats_outs = [[dram.tile([128, 2], F32, addr_space="Shared",
                                     name=f"stats_out_r{r}l{i}")
                           for i in range(L)] for r in range(reps)]
            gsum_in = dram.tile([B, H], F32, name="gsum_in")
            gsum_outs = [dram.tile([B, H], F32, addr_space="Shared",
                                   name=f"gsum_out_r{r}") for r in range(reps)]

            # ---- constants: iota row, identity, dis broadcast
            nc.gpsimd.iota(iota_i[:], pattern=[[1, 128]], base=0,
                           channel_multiplier=0)
            nc.vector.tensor_copy(iotaf[:], iota_i[:])
            nc.gpsimd.iota(iotap_i[:], pattern=[[0, 128]], base=0,
                           channel_multiplier=1)
            itp_f = perm.tile([128, 128], F32, name="itp_f")
            nc.vector.tensor_copy(itp_f[:], iotap_i[:])
            nc.vector.tensor_tensor(out=ident[:], in0=itp_f[:], in1=iotaf[:],
                                    op=ALU.is_equal)
            nc.vector.memset(ones1[:], 1.0)
            disrow_sb = perm.tile([1, NLOC], F32, name="disrow_sb")
            nc.sync.dma_start(disrow_sb[:], pview("disrow"))

            with tc.tile_pool(name="psB", bufs=2, space="PSUM") as psB:
                for g in range(NGRP):
                    w = 512 if g < NGRP - 1 else LASTP
                    pt = psB.tile([128, 512], F32, tag="pbc")
                    nc.tensor.matmul(pt[:, :w], lhsT=ones1[:],
                                     rhs=disrow_sb[:, g * 512: g * 512 + w],
                                     start=True, stop=True)
                    nc.vector.tensor_copy(disb[:, g * 512: g * 512 + w],
                                          pt[:, :w])

            for rep in range(reps):
                # ---- embed: h = relu(W_emb^T @ xT + b_emb)
                with (
                    tc.tile_pool(name="psE", bufs=2, space="PSUM") as psE,
                    tc.tile_pool(name="sbE", bufs=3) as sbE,
                ):
                    for g in range(NGRP):
                        w = 512 if g < NGRP - 1 else LASTP
                        xch = sbE.tile([F_NODE, 512], F32, tag="xch")
                        nc.sync.dma_start(
                            xch[:, :w],
                            pview("xt")[:, g * 512: g * 512 + w])
                        pt = psE.tile([128, 512], F32, tag="pse")
                        nc.tensor.matmul(pt[:, :w], lhsT=sb["wemb"][:],
                                         rhs=xch[:, :w],
                                         start=True, stop=True)
                        nc.scalar.activation(h[:, g * 512: g * 512 + w],
                                             pt[:, :w], AF.Relu,
                                             bias=sb["bemb"][:], scale=1.0)

                # ---- GCN layers
                for li in range(L):
                    # t = h @ W (node-major), scale by dis, cast bf16 -> stage
                    with tc.tile_pool(name=f"psT{li}", bufs=4, space="PSUM") as psT:
                        for nb in range(NBLK):
                            pn = 128 if nb < NBLK - 1 else LASTP
                            pt = psT.tile([128, H], F32, tag="pst")
                            nc.tensor.matmul(
                                pt[:pn, :],
                                lhsT=h[:, nb * 128: nb * 128 + pn],
                                rhs=sb["convw"][:, li * H: (li + 1) * H],
                                start=True, stop=True)
                            nc.scalar.activation(
                                stage[:pn, nb, :], pt[:pn, :], AF.Copy,
                                bias=0.0, scale=sb["disnm"][:pn, nb: nb + 1])
                    nc.sync.dma_start(
                        ag_in[0: 128 * (NBLK - 1), :].rearrange(
                            "(nb p) f -> p nb f", p=128),
                        stage[:, 0: NBLK - 1, :])
                    nc.sync.dma_start(ag_in[128 * (NBLK - 1): NLOC, :],
                                      stage[:LASTP, NBLK - 1, :])
                    table = tables[rep][li]
                    if not sim_mode:
                        nc.gpsimd.collective_compute(
                            "AllGather", ALU.bypass,
                            replica_groups=[list(range(NCORES))],
                            ins=[ag_in[:].opt()], outs=[table[:].opt()])

                    # scatter phase
                    with (
                        tc.tile_pool(name=f"msgp{li}", bufs=4) as msgp,
                        tc.tile_pool(name=f"psS{li}", bufs=2, space="PSUM") as psS,
                    ):
                        ci = 0
                        for g in range(NGRP):
                            w = 512 if g < NGRP - 1 else LASTP
                            psg = psS.tile([128, 512], F32, tag="psg")
                            nc.vector.memset(psg[:], 0.0)
                            while ci < len(calls) and calls[ci]["g"] == g:
                                cl = calls[ci]
                                nch = cl["nch"]
                                off = cl["off"]
                                last_call = (ci + 1 == len(calls)
                                             or calls[ci + 1]["g"] != g)
                                msg = msgp.tile([128, maxch, H], BF16, tag="msg")
                                src_ap = (table[0:SPLIT, :] if cl["grp"] == 0
                                          else table[BOFF:N, :])
                                gn = 1 if "gather" in skip else nch
                                nc.gpsimd.dma_gather(
                                    out_ap=msg[:, :gn, :], in_ap=src_ap,
                                    idxs_ap=sb["gidx"][:, off * 8: (off + gn) * 8],
                                    num_idxs=gn * 128, num_idxs_reg=gn * 128,
                                    elem_size=H)
                                oh = msgp.tile([128, maxch, 128], BF16, tag="oh")
                                d_b = sb["dloc"][:, off: off + nch].unsqueeze(
                                    2).broadcast_to([128, nch, 128])
                                i_b = iotaf[:].unsqueeze(1).broadcast_to(
                                    [128, nch, 128])
                                if "oh" in skip:
                                    d_b = d_b[:, :, :1]
                                    i_b = i_b[:, :, :1]
                                nc.vector.tensor_tensor(
                                    out=oh[:, :nch, :1] if "oh" in skip
                                    else oh[:, :nch, :],
                                    in0=d_b, in1=i_b, op=ALU.is_equal)
                                mmw = 1 if "mm" in skip else 128
                                for cc, t_rel in enumerate(cl["tl"]):
                                    nc.tensor.matmul(
                                        psg[:, t_rel * 128: t_rel * 128 + mmw],
                                        lhsT=msg[:, cc, :],
                                        rhs=oh[:, cc, :mmw],
                                        start=False,
                                        stop=(last_call and cc == nch - 1),
                                        skip_group_check=True)
                                ci += 1
                            # epilogue: x = psum * dis ; stats
                            cols = slice(g * 512, g * 512 + w)
                            nc.vector.tensor_tensor(out=xacc[:, cols],
                                                    in0=psg[:, :w],
                                                    in1=disb[:, cols],
                                                    op=ALU.mult)
                            nc.vector.tensor_reduce(
                                s1p[:, g: g + 1], xacc[:, cols],
                                axis=mybir.AxisListType.X, op=ALU.add)
                            nc.scalar.activation(scr[:, :w], xacc[:, cols],
                                                 AF.Square,
                                                 accum_out=s2p[:, g: g + 1])
                    # BN stats allreduce
                    nc.vector.tensor_reduce(sred[:, 0:1], s1p[:],
                                            axis=mybir.AxisListType.X, op=ALU.add)
                    nc.vector.tensor_reduce(sred[:, 1:2], s2p[:],
                                            axis=mybir.AxisListType.X, op=ALU.add)
                    nc.sync.dma_start(stats_in[:], sred[:])
                    stats_out = stats_outs[rep][li]
                    if not sim_mode:
                        nc.gpsimd.collective_compute(
                            "AllReduce", ALU.add,
                            replica_groups=[list(range(NCORES))],
                            ins=[stats_in[:].opt()], outs=[stats_out[:].opt()])
                        nc.sync.dma_start(statr[:], stats_out[:])
                    else:
                        nc.sync.dma_start(statr[:], stats_in[:])
                    # scale/shift
                    nc.vector.tensor_scalar_mul(bnc[:, 0:1], statr[:, 0:1], 1.0 / N)
                    nc.vector.tensor_scalar_mul(bnc[:, 1:2], statr[:, 1:2], 1.0 / N)
                    nc.vector.tensor_tensor(out=bnc[:, 2:3], in0=bnc[:, 0:1],
                                            in1=bnc[:, 0:1], op=ALU.mult)
                    nc.vector.tensor_tensor(out=bnc[:, 3:4], in0=bnc[:, 1:2],
                                            in1=bnc[:, 2:3], op=ALU.subtract)
                    nc.vector.tensor_scalar_add(bnc[:, 3:4], bnc[:, 3:4], EPS)
                    nc.scalar.sqrt(bnc[:, 4:5], bnc[:, 3:4])
                    nc.vector.reciprocal(bnc[:, 5:6], bnc[:, 4:5])
                    nc.vector.tensor_tensor(out=bnc[:, 6:7],
                                            in0=sb["bng"][:, li: li + 1],
                                            in1=bnc[:, 5:6], op=ALU.mult)
                    nc.vector.tensor_tensor(out=bnc[:, 7:8], in0=bnc[:, 0:1],
                                            in1=bnc[:, 6:7], op=ALU.mult)
                    nc.vector.tensor_tensor(out=bnc[:, 7:8],
                                            in0=sb["bnb"][:, li: li + 1],
                                            in1=bnc[:, 7:8], op=ALU.subtract)
                    # apply BN + relu (+ residual)
                    for g in range(NGRP):
                        w = 512 if g < NGRP - 1 else LASTP
                        cols = slice(g * 512, g * 512 + w)
                        if li == 0:
                            nc.scalar.activation(h[:, cols], xacc[:, cols],
                                                 AF.Relu, bias=bnc[:, 7:8],
                                                 scale=bnc[:, 6:7])
                        else:
                            nc.scalar.activation(scr2[:, :w], xacc[:, cols],
                                                 AF.Relu, bias=bnc[:, 7:8],
                                                 scale=bnc[:, 6:7])
                            nc.vector.tensor_tensor(out=h[:, cols],
                                                    in0=scr2[:, :w],
                                                    in1=h[:, cols], op=ALU.add)
                    if debug_taps:
                        nc.sync.dma_start(taps[li][:], h[:])

                # ---- global mean pool
                with (
                    tc.tile_pool(name="psP", bufs=4, space="PSUM") as psP,
                    tc.tile_pool(name="sbP", bufs=1) as sbP,
                ):
                    ohb = sbP.tile([128, NBLK, B], F32, tag="ohb")
                    gb_b = sb["gblk"][:].unsqueeze(2).broadcast_to(
                        [128, NBLK, B])
                    io_b = iotaf[:, :B].unsqueeze(1).broadcast_to(
                        [128, NBLK, B])
                    nc.vector.tensor_tensor(out=ohb[:], in0=gb_b, in1=io_b,
                                            op=ALU.is_equal)
                    gps = psP.tile([B, H], F32, tag="gps", bufs=1)
                    nc.vector.memset(gps[:], 0.0)
                    for nb in range(NBLK):
                        pn = 128 if nb < NBLK - 1 else LASTP
                        tr = psP.tile([128, 128], F32, tag="ptr")
                        nc.tensor.transpose(tr[:pn, :],
                                            h[:, nb * 128: nb * 128 + pn],
                                            ident[:])
                        nc.scalar.activation(h3nm[:pn, :], tr[:pn, :], AF.Copy,
                                             bias=0.0, scale=1.0)
                        nc.tensor.matmul(gps[:], lhsT=ohb[:pn, nb, :],
                                         rhs=h3nm[:pn, :], start=False,
                                         stop=(nb == NBLK - 1),
                                         skip_group_check=True)
                    nc.scalar.activation(gsum_sb[:], gps[:], AF.Copy,
                                         bias=0.0, scale=1.0)
                nc.sync.dma_start(gsum_in[:], gsum_sb[:])
                gsum_out = gsum_outs[rep]
                if not sim_mode:
                    nc.gpsimd.collective_compute(
                        "AllReduce", ALU.add,
                        replica_groups=[list(range(NCORES))],
                        ins=[gsum_in[:].opt()], outs=[gsum_out[:].opt()])
                gsumr = perm.tile([B, H], F32, name=f"gsumr{rep}", tag="gsumr")
                nc.sync.dma_start(gsumr[:], gsum_out[:] if not sim_mode
                                  else gsum_in[:])

                # ---- final MLP (replicated on every core)
                with tc.tile_pool(name="psM", bufs=1, space="PSUM") as psM:
                    gemb = perm.tile([B, H], F32, name=f"gemb{rep}", tag="gemb")
                    nc.vector.tensor_scalar_mul(gemb[:], gsumr[:],
                                                sb["invcnt"][:])
                    ptr = psM.tile([H, B], F32, tag="mtr")
                    nc.tensor.transpose(ptr[:], gemb[:], ident[:B, :B])
                    nc.scalar.activation(gembT[:], ptr[:], AF.Copy, bias=0.0,
                                         scale=1.0)
                    # gf path
                    p1 = psM.tile([H // 2, B], F32, tag="m1")
                    nc.tensor.matmul(p1[:], lhsT=sb["gfw1"][:], rhs=sb["gft"][:],
                                     start=True, stop=True)
                    g1s = perm.tile([H // 2, B], F32, name=f"g1s{rep}", tag="g1s")
                    nc.scalar.activation(g1s[:], p1[:], AF.Relu,
                                         bias=sb["gfb1"][:], scale=1.0)
                    p2 = psM.tile([H // 4, B], F32, tag="m2")
                    nc.tensor.matmul(p2[:], lhsT=sb["gfw2"][:], rhs=g1s[:],
                                     start=True, stop=True)
                    g2s = perm.tile([H // 4, B], F32, name=f"g2s{rep}", tag="g2s")
                    nc.scalar.activation(g2s[:], p2[:], AF.Identity,
                                         bias=sb["gfb2"][:], scale=1.0)
                    # head
                    q1 = psM.tile([H // 2, B], F32, tag="m3")
                    nc.tensor.matmul(q1[:], lhsT=sb["pw1a"][:], rhs=gembT[:],
                                     start=True, stop=False,
                                     skip_group_check=True)
                    nc.tensor.matmul(q1[:], lhsT=sb["pw1b"][:], rhs=g2s[:],
                                     start=False, stop=True,
                                     skip_group_check=True)
                    q1s = perm.tile([H // 2, B], F32, name=f"q1s{rep}", tag="q1s")
                    nc.scalar.activation(q1s[:], q1[:], AF.Relu,
                                         bias=sb["pb1"][:], scale=1.0)
                    q2 = psM.tile([H // 4, B], F32, tag="m4")
                    nc.tensor.matmul(q2[:], lhsT=sb["pw2"][:], rhs=q1s[:],
                                     start=True, stop=True)
                    q2s = perm.tile([H // 4, B], F32, name=f"q2s{rep}", tag="q2s")
                    nc.scalar.activation(q2s[:], q2[:], AF.Relu,
                                         bias=sb["pb2"][:], scale=1.0)
                    q3 = psM.tile([1, B], F32, tag="m5")
                    nc.tensor.matmul(q3[:], lhsT=sb["pw3"][:], rhs=q2s[:],
                                     start=True, stop=True)
                    nc.scalar.activation(outsb[:], q3[:], AF.Identity,
                                         bias=sb["pb3"][:], scale=1.0)
                nc.sync.dma_start(out_p[:].rearrange("b o -> o b"), outsb[:])

    nc.compile()
    return nc


def kernel(**inputs):
    plan, in_maps = build_plan(**inputs)
    nc = build_nc(plan)
    res = run_bass_kernel_spmd(nc, in_maps, core_ids=list(range(NCORES)),
                               trace=False)
    return np.asarray(res.results[0]["out"], np.float32)
